# revision 12
# baseline (speedup 1.0000x reference)
"""Distributed Trainium2 Bass kernel for BlockchainGNN (2xSAGE + GAT + MLP).

Sharding: nodes are partitioned across 8 cores (6250 each). Edges are owned
by the dst core, sorted by dst into 128-edge matmul chunks per 128-node dst
tile. Gathers of node features run via dma_gather (SWDGE descriptor gather;
int16 indices, so every gather source table is split in two 25000-row
halves and edges are grouped per half). Scatter-add (segment_sum) runs on
the TensorEngine as one-hot matmuls: psum[dst_tile, C] += M_c.T @ xg_c
where M_c[e, d] = 1 iff edge e of chunk c targets dst d of the tile. M is
built on the DVE via iota/is_equal from host-provided per-edge dst offsets
(-1 marks padding -> all-zero row -> exact no-op). Gather tables and matmul
operands are bf16 (psum accumulation stays f32). Features move between
layers through AllGather. GAT softmax: the denominator is factored out of
the edge scatter (exact in real arithmetic); self-loops are handled
analytically per node.
"""

import os
import sys

sys.path.insert(0, "/opt/trn_rl_repo")

import ml_dtypes
import numpy as np

import concourse.bacc as bacc
import concourse.mybir as mybir
import concourse.tile as tile
from concourse.tile_rust import add_dep_helper
from concourse.bass_utils import run_bass_kernel_spmd

N = 50000
E = 800000
P = 8
NLOC = N // P          # 6250
HALF = N // 2          # 25000
IN_C, HID, OUT_C, HEADS = 64, 128, 32, 4
BN_EPS = 1e-5
NEG_SLOPE = 0.2
NT = (NLOC + 127) // 128            # 49 dst tiles per core
LAST_ROWS = NLOC - (NT - 1) * 128   # 106 rows in the last tile
TB_L = 3    # dst tiles per gather batch, SAGE layers
TB_G = 2    # dst tiles per gather batch, GAT layer
XW = 128    # xbf table row width (x 64 | pad), bf16 -> 256B rows
GATW = 256  # gat table row width (g 128 | a_src 4 | pad), bf16 -> 512B
LOCW = 128  # local table row width (a_dst in cols 0:4), bf16 -> 256B

f32 = mybir.dt.float32
bf16 = mybir.dt.bfloat16
i16 = mybir.dt.int16


def _wrap_idx(vals: np.ndarray) -> np.ndarray:
    """[n] int -> [128, n//16] int16: idx i at [i%16, i//16], replicated to
    128 partitions (8 Q7 cores x 16 partitions each)."""
    n = len(vals)
    assert n % 16 == 0
    w = vals.astype(np.int16).reshape(n // 16, 16).T
    return np.ascontiguousarray(np.tile(w, (8, 1)))


def preprocess(edge_index: np.ndarray):
    """Build the shared chunk layout + per-core index arrays."""
    src = np.asarray(edge_index[0]).astype(np.int64)
    dst = np.asarray(edge_index[1]).astype(np.int64)
    owner = dst // NLOC

    counts = np.zeros((P, 2, NT), dtype=np.int64)
    per_core_edges = []
    for c in range(P):
        m = owner == c
        s_c = src[m]
        d_c = dst[m] - c * NLOC
        halves = []
        for h in range(2):
            hm = (s_c < HALF) if h == 0 else (s_c >= HALF)
            ss = s_c[hm] - h * HALF
            dd = d_c[hm]
            order = np.argsort(dd, kind="stable")
            ss, dd = ss[order], dd[order]
            t_of = dd // 128
            halves.append((ss, dd, t_of))
            counts[c, h] = np.bincount(t_of, minlength=NT)
        per_core_edges.append(halves)

    nch = (counts.max(axis=0) + 127) // 128  # [2, NT]
    koff = np.zeros((2, NT + 1), dtype=np.int64)
    for h in range(2):
        koff[h, 1:] = np.cumsum(nch[h])
    eg = koff[:, -1] * 128

    per_core = []
    for c in range(P):
        data = {}
        for h in range(2):
            ss, dd, t_of = per_core_edges[c][h]
            gi = np.zeros(eg[h], dtype=np.int16)
            gd = np.zeros(eg[h], dtype=np.int16)
            dso = np.full(eg[h], -1.0, dtype=np.float32)
            for t in range(NT):
                sel = t_of == t
                k = int(sel.sum())
                base = koff[h, t] * 128
                gi[base : base + k] = ss[sel]
                gd[base : base + k] = dd[sel]
                dso[base : base + k] = (dd[sel] - t * 128).astype(np.float32)
            data[f"gi{h}"] = _wrap_idx(gi)
            data[f"gd{h}"] = _wrap_idx(gd)
            data[f"dso{h}"] = np.ascontiguousarray(
                dso.reshape(-1, 128).T.astype(ml_dtypes.bfloat16))
        per_core.append(data)

    return {"nch": nch, "koff": koff, "eg": eg}, per_core


def _batches(tb):
    t = 0
    while t < NT:
        yield t, min(tb, NT - t)
        t += tb


def build(layout) -> bacc.Bacc:
    nch, koff, eg = layout["nch"], layout["koff"], layout["eg"]

    nc = bacc.Bacc("TRN2", target_bir_lowering=False, debug=False,
                   num_devices=P)

    d_xbf = nc.declare_dram_parameter("xbf", [N, XW], bf16, isOutput=False)
    d_xloc = nc.declare_dram_parameter("xloc", [NLOC, IN_C], f32,
                                       isOutput=False)
    dp = {}
    for name, shape in [
        ("sage1_wl", [IN_C, HID]), ("sage1_wr", [IN_C, HID]),
        ("sage2_wl", [HID, HID]), ("sage2_wr", [HID, HID]),
        ("gat_w", [HID, HID]),
        ("cls_w1", [OUT_C, 64]), ("cls_w2", [64, 1]),
        ("sage1_bl", [1, HID]), ("sage2_bl", [1, HID]),
        ("bn1_gamma", [1, HID]), ("bn1_beta", [1, HID]),
        ("bn1_mean", [1, HID]), ("bn1_var", [1, HID]),
        ("bn2_gamma", [1, HID]), ("bn2_beta", [1, HID]),
        ("bn2_mean", [1, HID]), ("bn2_var", [1, HID]),
        ("att_src_row", [1, HID]), ("att_dst_row", [1, HID]),
        ("gat_bias", [1, OUT_C]), ("cls_b1", [1, 64]), ("cls_b2", [1, 1]),
        ("ident", [128, 128]),
    ]:
        dp[name] = nc.declare_dram_parameter(name, shape, f32, isOutput=False)
    dp["iotaf"] = nc.declare_dram_parameter("iotaf", [128, 128], bf16,
                                            isOutput=False)
    for h in range(2):
        dp[f"gi{h}"] = nc.declare_dram_parameter(
            f"gi{h}", [128, int(eg[h]) // 16], i16, isOutput=False)
        dp[f"gd{h}"] = nc.declare_dram_parameter(
            f"gd{h}", [128, int(eg[h]) // 16], i16, isOutput=False)
        dp[f"dso{h}"] = nc.declare_dram_parameter(
            f"dso{h}", [128, int(koff[h, -1])], bf16, isOutput=False)
    d_emb = nc.declare_dram_parameter("emb", [NLOC, OUT_C], f32,
                                      isOutput=True)
    d_log = nc.declare_dram_parameter("logits", [NLOC, 1], f32,
                                      isOutput=True)

    h1loc = nc.dram_tensor("h1loc", [NLOC, HID], bf16)
    h1full = nc.dram_tensor("h1full", [N, HID], bf16, addr_space="Shared")
    gtloc = nc.dram_tensor("gtloc", [NLOC, GATW], bf16)
    gtfull = nc.dram_tensor("gtfull", [N, GATW], bf16, addr_space="Shared")
    loctab = nc.dram_tensor("loctab", [NLOC, LOCW], bf16)

    RG = [list(range(P))]
    AF = mybir.ActivationFunctionType

    with tile.TileContext(nc) as tc:
        with (
            tc.tile_pool(name="const", bufs=1) as cpool,
            tc.tile_pool(name="work", bufs=3) as wpool,
            tc.tile_pool(name="gath", bufs=3) as gpool,
            tc.tile_pool(name="mpool", bufs=3) as mpool,
            tc.tile_pool(name="pagg", bufs=2, space="PSUM") as pagg,
            tc.tile_pool(name="psmall", bufs=2, space="PSUM") as psmall,
            tc.tile_pool(name="pdense", bufs=2, space="PSUM") as pdense,
            tc.tile_pool(name="ptrans", bufs=2, space="PSUM") as ptrans,
        ):
            def load_const(name, shape, dt=f32):
                t = cpool.tile(shape, dt, tag=name, name=name)
                nc.sync.dma_start(t[:], dp[name][:])
                return t

            def load_const_bf(name, shape):
                stage = wpool.tile(shape, f32, tag="stage", name="stage")
                nc.sync.dma_start(stage[:], dp[name][:])
                t = cpool.tile(shape, bf16, tag=name, name=name)
                nc.vector.tensor_copy(t[:], stage[:])
                return t

            wl1 = load_const_bf("sage1_wl", [IN_C, HID])
            wr1 = load_const_bf("sage1_wr", [IN_C, HID])
            wl2 = load_const_bf("sage2_wl", [HID, HID])
            wr2 = load_const_bf("sage2_wr", [HID, HID])
            gatw = load_const_bf("gat_w", [HID, HID])
            w1 = load_const_bf("cls_w1", [OUT_C, 64])
            w2 = load_const_bf("cls_w2", [64, 1])
            ident = load_const("ident", [128, 128])
            ident_bf = cpool.tile([128, 128], bf16, tag="ident_bf",
                                  name="ident_bf")
            nc.vector.tensor_copy(ident_bf[:], ident[:])
            iotaf = load_const("iotaf", [128, 128], bf16)
            rows = {}
            for nm in ["sage1_bl", "sage2_bl", "bn1_gamma", "bn1_beta",
                       "bn1_mean", "bn1_var", "bn2_gamma", "bn2_beta",
                       "bn2_mean", "bn2_var", "att_src_row", "att_dst_row"]:
                rows[nm] = load_const(nm, [1, HID])
            gbias = load_const("gat_bias", [1, OUT_C])
            b1 = load_const("cls_b1", [1, 64])
            b2 = load_const("cls_b2", [1, 1])

            gidx = {}
            for h in range(2):
                for pre in ("gi", "gd"):
                    t = cpool.tile([128, int(eg[h]) // 16], i16,
                                   tag=f"{pre}{h}", name=f"{pre}{h}")
                    nc.sync.dma_start(t[:], dp[f"{pre}{h}"][:])
                    gidx[f"{pre}{h}"] = t
                t = cpool.tile([128, int(koff[h, -1])], bf16, tag=f"dso{h}",
                               name=f"dso{h}")
                nc.sync.dma_start(t[:], dp[f"dso{h}"][:])
                gidx[f"dso{h}"] = t

            ones = cpool.tile([128, 1], bf16, tag="ones", name="ones")
            nc.vector.memset(ones[:], 1.0)

            def bn_fold(pref, bl):
                v = cpool.tile([1, HID], f32, tag=f"{pref}_v",
                               name=f"{pref}_v")
                nc.vector.tensor_scalar_add(v[:], rows[f"{pref}_var"][:],
                                            BN_EPS)
                r = cpool.tile([1, HID], f32, tag=f"{pref}_r",
                               name=f"{pref}_r")
                nc.vector.reciprocal(r[:], v[:])
                s = cpool.tile([1, HID], f32, tag=f"{pref}_s",
                               name=f"{pref}_s")
                nc.scalar.sqrt(s[:], r[:])
                scale = cpool.tile([1, HID], f32, tag=f"{pref}_scale",
                                   name=f"{pref}_scale")
                nc.vector.tensor_mul(scale[:], s[:], rows[f"{pref}_gamma"][:])
                t0 = cpool.tile([1, HID], f32, tag=f"{pref}_t0",
                                name=f"{pref}_t0")
                nc.vector.tensor_sub(t0[:], rows[bl][:],
                                     rows[f"{pref}_mean"][:])
                nc.vector.tensor_mul(t0[:], t0[:], scale[:])
                bias = cpool.tile([1, HID], f32, tag=f"{pref}_bias",
                                  name=f"{pref}_bias")
                nc.vector.tensor_add(bias[:], rows[f"{pref}_beta"][:], t0[:])
                return scale, bias

            scale1, bias1 = bn_fold("bn1", "sage1_bl")
            scale2, bias2 = bn_fold("bn2", "sage2_bl")

            ones1 = cpool.tile([1, 128], f32, tag="ones1", name="ones1")
            nc.vector.memset(ones1[:], 1.0)

            def bcast_row(row, w, name):
                """[1, w] -> [128, w] via ones-column outer product on PE."""
                pbt = ptrans.tile([128, w], f32, tag="ptrans", name="ptrans")
                nc.tensor.matmul(pbt[:], ones1[:], row, start=True, stop=True)
                out = cpool.tile([128, w], f32, tag=name, name=name)
                nc.vector.tensor_copy(out[:], pbt[:])
                return out

            if int(os.environ.get("KSETUP", "2")) < 1:
                def bcast_row(row, w, name):
                    out = cpool.tile([128, w], f32, tag=name, name=name)
                    nc.vector.memset(out[:], 0.0)
                    return out
            scale1b = bcast_row(scale1[:], HID, "scale1b")
            bias1b = bcast_row(bias1[:], HID, "bias1b")
            scale2b = bcast_row(scale2[:], HID, "scale2b")
            bias2b = bcast_row(bias2[:], HID, "bias2b")
            attsb = bcast_row(rows["att_src_row"][:], HID, "attsb")
            attdb = bcast_row(rows["att_dst_row"][:], HID, "attdb")
            gbiasb = bcast_row(gbias[:], OUT_C, "gbiasb")
            b1b = bcast_row(b1[:], 64, "b1b")
            b2b = bcast_row(b2[:], 1, "b2b")

            rdeg_all = cpool.tile([128, NT], f32, tag="rdeg", name="rdeg")
            asrc_all = cpool.tile([128, NT * HEADS], f32, tag="asrc",
                                  name="asrc")
            adst_all = cpool.tile([128, NT * HEADS], f32, tag="adst",
                                  name="adst")
            exself_all = cpool.tile([128, NT * HEADS], f32, tag="exself",
                                    name="exself")

            def rows_of(t):
                return LAST_ROWS if t == NT - 1 else 128

            def pb(row_ap, n=128):
                return row_ap.partition_broadcast(n)

            def build_m(h, k0, nb):
                m = mpool.tile([128, nb, 128], bf16, tag="m", name="m")
                in0 = iotaf[:].unsqueeze(1).broadcast_to([128, nb, 128])
                in1 = gidx[f"dso{h}"][:, k0 : k0 + nb].unsqueeze(2) \
                    .broadcast_to([128, nb, 128])
                nc.vector.tensor_tensor(m[:], in0, in1,
                                        mybir.AluOpType.is_equal)
                return m

            gather_reg = []

            def gather(src_ap, h, k0, nb, width, idx, tag):
                buf = gpool.tile([128, nb, width], bf16, tag=tag, name=tag)
                ixt = gidx[f"{idx}{h}"]
                for o in range(0, nb, 8):
                    n2 = min(8, nb - o)
                    gi = nc.gpsimd.dma_gather(
                        buf[:, o : o + n2, :], src_ap,
                        ixt[:, (k0 + o) * 8 : (k0 + o + n2) * 8],
                        n2 * 128, n2 * 128, width)
                    gather_reg.append(gi.ins)
                return buf

            def transpose_to(sb_in, width, tag="tp"):
                pt = ptrans.tile([width, 128], sb_in.dtype, tag="ptrans",
                                 name="ptrans")
                idn = ident if sb_in.dtype == f32 else ident_bf
                nc.tensor.transpose(pt[:], sb_in, idn[:])
                out = wpool.tile([width, 128], bf16, tag=f"{tag}{width}",
                                 name=f"{tag}{width}")
                nc.scalar.copy(out[:], pt[:])
                return out

            # ======================= SAGE layers =========================
            def sage(layer, src_halves, width, wl, wr, scaleb, biasb):
                ksage = int(os.environ.get("KSAGE", "4"))
                for t0, ntile in _batches(TB_L):
                    bufs, ms = {}, {}
                    for h in range(2):
                        k0 = int(koff[h, t0])
                        nb = int(koff[h, t0 + ntile] - k0)
                        if nb == 0:
                            continue
                        bufs[h] = (k0, nb,
                                   gather(src_halves[h], h, k0, nb, XW
                                          if layer == 1 else HID, "gi", "sg"))
                        if ksage >= 2:
                            ms[h] = build_m(h, k0, nb)
                    if ksage < 3:
                        continue
                    for ti in range(ntile):
                        t = t0 + ti
                        tot = int(nch[0][t] + nch[1][t])
                        pa = pagg.tile([128, width], f32, tag="pagg",
                                       name="pagg")
                        pd = psmall.tile([128, 1], f32, tag="psmall",
                                         name="psmall") \
                            if layer == 1 else None
                        done = 0
                        for h in range(2):
                            if h not in bufs:
                                continue
                            k0, nb, buf = bufs[h]
                            m = ms[h]
                            c0 = int(koff[h, t] - k0)
                            for c in range(int(nch[h][t])):
                                lc = c0 + c
                                first = done == 0
                                last = done == tot - 1
                                nc.tensor.matmul(
                                    pa[:], m[:, lc, :],
                                    buf[:, lc, 0:width],
                                    start=first, stop=last)
                                if layer == 1:
                                    nc.tensor.matmul(
                                        pd[:], m[:, lc, :], ones[:],
                                        start=first, stop=last)
                                done += 1
                        # ---- per-tile epilogue ----
                        if ksage < 4:
                            continue
                        r = rows_of(t)
                        if layer == 1:
                            dg = wpool.tile([128, 1], f32, tag="deg",
                                            name="deg")
                            nc.vector.tensor_scalar_max(dg[:], pd[:], 1.0)
                            nc.vector.reciprocal(rdeg_all[:, t : t + 1],
                                                 dg[:])
                        mean = wpool.tile([128, width], f32, tag="mean",
                                          name="mean")
                        nc.scalar.activation(mean[:], pa[:], AF.Copy,
                                             scale=rdeg_all[:, t : t + 1])
                        meanT = transpose_to(mean[:], width, "mT")
                        slf = wpool.tile([128, width],
                                         f32 if layer == 1 else bf16,
                                         tag=f"slf{layer}",
                                         name=f"slf{layer}")
                        if layer == 1:
                            nc.sync.dma_start(
                                slf[:r, :], d_xloc[t * 128 : t * 128 + r, :])
                        else:
                            nc.sync.dma_start(
                                slf[:r, :], h1loc[t * 128 : t * 128 + r, :])
                        selfT = transpose_to(slf[:], width, "sT")
                        ph = pdense.tile([128, HID], f32, tag="pdense",
                                         name="pdense")
                        nc.tensor.matmul(ph[:], meanT[:], wl[:],
                                         start=True, stop=False)
                        nc.tensor.matmul(ph[:], selfT[:], wr[:],
                                         start=False, stop=True)
                        tmp = wpool.tile([128, HID], f32, tag="bn",
                                         name="bn")
                        nc.vector.tensor_mul(tmp[:], ph[:], scaleb[:])
                        nc.vector.tensor_add(tmp[:], tmp[:], biasb[:])
                        if layer == 2:
                            nc.vector.tensor_add(tmp[:], tmp[:], slf[:])
                        hout = wpool.tile([128, HID],
                                          bf16 if layer == 1 else f32,
                                          tag=f"hout{layer}",
                                          name=f"hout{layer}")
                        nc.scalar.activation(hout[:], tmp[:], AF.Relu)
                        if layer == 1:
                            nc.sync.dma_start(
                                h1loc[t * 128 : t * 128 + r, :], hout[:r])
                        else:
                            gat_local(t, hout)

            def gat_local(t, h2t):
                r = rows_of(t)
                h2T = transpose_to(h2t[:], HID, "h2T")
                pg = pdense.tile([128, HID], f32, tag="pdense",
                                 name="pdense")
                nc.tensor.matmul(pg[:], h2T[:], gatw[:], start=True,
                                 stop=True)
                g = wpool.tile([128, HID], f32, tag="g", name="g")
                nc.vector.tensor_copy(g[:], pg[:])
                gb = wpool.tile([128, HID], bf16, tag="gb", name="gb")
                nc.scalar.copy(gb[:], pg[:])
                s4 = slice(t * HEADS, (t + 1) * HEADS)
                for row, dst_t in ((attsb, asrc_all), (attdb, adst_all)):
                    tmp = wpool.tile([128, HID], f32, tag="att", name="att")
                    nc.vector.tensor_mul(tmp[:], g[:], row[:])
                    nc.vector.tensor_reduce(
                        dst_t[:, s4],
                        tmp[:].rearrange("p (h c) -> p h c", h=HEADS),
                        mybir.AxisListType.X, mybir.AluOpType.add)
                es = wpool.tile([128, HEADS], f32, tag="es", name="es")
                nc.vector.tensor_add(es[:], asrc_all[:, s4], adst_all[:, s4])
                nc.scalar.activation(es[:], es[:], AF.Lrelu, alpha=NEG_SLOPE)
                nc.scalar.activation(exself_all[:, s4], es[:], AF.Exp)
                ab = wpool.tile([128, 2 * HEADS], bf16, tag="ab", name="ab")
                nc.vector.tensor_copy(ab[:, 0:HEADS], asrc_all[:, s4])
                nc.vector.tensor_copy(ab[:, HEADS : 2 * HEADS],
                                      adst_all[:, s4])
                nc.sync.dma_start(gtloc[t * 128 : t * 128 + r, 0:HID],
                                  gb[:r])
                nc.sync.dma_start(
                    gtloc[t * 128 : t * 128 + r, HID : HID + HEADS],
                    ab[:r, 0:HEADS])
                w = nc.sync.dma_start(loctab[t * 128 : t * 128 + r, 0:HEADS],
                                      ab[:r, HEADS : 2 * HEADS])
                loctab_writers.append(w.ins)

            # ---- pipeline ----
            loctab_writers = []
            stage = int(os.environ.get("KSTAGE", "4"))
            if stage >= 1:
                gather_reg.clear()
                sage(1, (d_xbf[0:HALF, :], d_xbf[HALF:, :]), IN_C, wl1, wr1,
                     scale1b, bias1b)
            if stage >= 2:
                ag1 = nc.gpsimd.collective_compute(
                    "AllGather", mybir.AluOpType.bypass, replica_groups=RG,
                    ins=[h1loc[:].opt()], outs=[h1full[:].opt()])
            if stage >= 3:
                gather_reg.clear()
                sage(2, (h1full[0:HALF, :], h1full[HALF:, :]), HID, wl2,
                     wr2, scale2b, bias2b)
                for g in gather_reg:
                    add_dep_helper(g, ag1.ins, True, "gather waits h1full AG")
                ag2 = nc.gpsimd.collective_compute(
                    "AllGather", mybir.AluOpType.bypass, replica_groups=RG,
                    ins=[gtloc[:].opt()], outs=[gtfull[:].opt()])
                gather_reg.clear()

            # ======================= GAT edge phase ======================
            for t0, ntile in (_batches(TB_G) if stage >= 4 else []):
                bufs, ms, exs = {}, {}, {}
                for h in range(2):
                    k0 = int(koff[h, t0])
                    nb = int(koff[h, t0 + ntile] - k0)
                    if nb == 0:
                        continue
                    n0 = len(gather_reg)
                    ga = gather(
                        gtfull[0:HALF, :] if h == 0 else gtfull[HALF:, :],
                        h, k0, nb, GATW, "gi", "ga")
                    for g in gather_reg[n0:]:
                        add_dep_helper(g, ag2.ins, True, "ga waits gtfull AG")
                    n0 = len(gather_reg)
                    gd = gather(loctab[:], h, k0, nb, LOCW, "gd", "gd")
                    for g in gather_reg[n0:]:
                        for w in loctab_writers:
                            add_dep_helper(g, w, True, "gd waits loctab write")
                    ex = gpool.tile([128, nb, HEADS], bf16, tag="ex",
                                    name="ex")
                    nc.vector.tensor_add(ex[:],
                                         ga[:, :, HID : HID + HEADS],
                                         gd[:, :, 0:HEADS])
                    nc.scalar.activation(ex[:], ex[:], AF.Lrelu,
                                         alpha=NEG_SLOPE)
                    nc.scalar.activation(ex[:], ex[:], AF.Exp)
                    gview = ga[:, :, 0:HID].rearrange(
                        "p n (h c) -> p n h c", h=HEADS)
                    exb = ex[:].unsqueeze(3).broadcast_to(
                        [128, nb, HEADS, HID // HEADS])
                    nc.vector.tensor_mul(gview, gview, exb)
                    bufs[h] = (k0, nb, ga)
                    ms[h] = build_m(h, k0, nb)
                    exs[h] = ex
                for ti in range(ntile):
                    t = t0 + ti
                    tot = int(nch[0][t] + nch[1][t])
                    pw = pagg.tile([128, HID], f32, tag="pagg", name="pagg")
                    pden = psmall.tile([128, HEADS], f32, tag="psmall",
                                       name="psmall")
                    done = 0
                    for h in range(2):
                        if h not in bufs:
                            continue
                        k0, nb, ga = bufs[h]
                        c0 = int(koff[h, t] - k0)
                        for c in range(int(nch[h][t])):
                            lc = c0 + c
                            first = done == 0
                            last = done == tot - 1
                            nc.tensor.matmul(pw[:], ms[h][:, lc, :],
                                             ga[:, lc, 0:HID],
                                             start=first, stop=last)
                            nc.tensor.matmul(pden[:], ms[h][:, lc, :],
                                             exs[h][:, lc, :],
                                             start=first, stop=last)
                            done += 1
                    # ---- per-tile epilogue ----
                    r = rows_of(t)
                    s4 = slice(t * HEADS, (t + 1) * HEADS)
                    g = wpool.tile([128, HID], bf16, tag="g2", name="g2")
                    nc.sync.dma_start(g[:r, :],
                                      gtloc[t * 128 : t * 128 + r, 0:HID])
                    den = wpool.tile([128, HEADS], f32, tag="den",
                                     name="den")
                    nc.vector.tensor_add(den[:], pden[:], exself_all[:, s4])
                    rden = wpool.tile([128, HEADS], f32, tag="rden",
                                      name="rden")
                    nc.vector.reciprocal(rden[:], den[:])
                    wsum = wpool.tile([128, HID], f32, tag="wsum",
                                      name="wsum")
                    exb = exself_all[:, s4].unsqueeze(2).broadcast_to(
                        [128, HEADS, HID // HEADS])
                    nc.vector.tensor_mul(
                        wsum[:].rearrange("p (h c) -> p h c", h=HEADS),
                        g[:].rearrange("p (h c) -> p h c", h=HEADS), exb)
                    nc.vector.tensor_add(wsum[:], wsum[:], pw[:])
                    rdenb = rden[:].unsqueeze(2).broadcast_to(
                        [128, HEADS, HID // HEADS])
                    nc.vector.tensor_mul(
                        wsum[:].rearrange("p (h c) -> p h c", h=HEADS),
                        wsum[:].rearrange("p (h c) -> p h c", h=HEADS),
                        rdenb)
                    s01 = wpool.tile([128, OUT_C], f32, tag="s01",
                                     name="s01")
                    nc.vector.tensor_add(s01[:], wsum[:, 0:32],
                                         wsum[:, 32:64])
                    s23 = wpool.tile([128, OUT_C], f32, tag="s23",
                                     name="s23")
                    nc.vector.tensor_add(s23[:], wsum[:, 64:96],
                                         wsum[:, 96:128])
                    emb = wpool.tile([128, OUT_C], f32, tag="emb",
                                     name="emb")
                    nc.vector.tensor_add(emb[:], s01[:], s23[:])
                    nc.vector.tensor_scalar_mul(emb[:], emb[:], 0.25)
                    nc.vector.tensor_add(emb[:], emb[:], gbiasb[:])
                    nc.sync.dma_start(d_emb[t * 128 : t * 128 + r, :],
                                      emb[:r])
                    # ---- MLP ----
                    embT = transpose_to(emb[:], OUT_C, "eT")
                    pz1 = pdense.tile([128, 64], f32, tag="pdense",
                                      name="pdense")
                    nc.tensor.matmul(pz1[:], embT[:], w1[:], start=True,
                                     stop=True)
                    r1 = wpool.tile([128, 64], f32, tag="r1", name="r1")
                    nc.vector.tensor_add(r1[:], pz1[:], b1b[:])
                    nc.scalar.activation(r1[:], r1[:], AF.Relu)
                    r1T = transpose_to(r1[:], 64, "rT")
                    pz2 = pdense.tile([128, 1], f32, tag="pdense",
                                      name="pz2")
                    nc.tensor.matmul(pz2[:], r1T[:], w2[:], start=True,
                                     stop=True)
                    lg = wpool.tile([128, 1], f32, tag="lg", name="lg")
                    nc.vector.tensor_add(lg[:], pz2[:], b2b[:])
                    nc.sync.dma_start(d_log[t * 128 : t * 128 + r, :],
                                      lg[:r])

    nc.compile()
    return nc


def make_in_maps(inputs, per_core):
    xf = np.asarray(inputs["x"], dtype=np.float32)
    xbf = np.zeros((N, XW), dtype=ml_dtypes.bfloat16)
    xbf[:, :IN_C] = xf.astype(ml_dtypes.bfloat16)
    base = {
        "xbf": xbf,
        "ident": np.eye(128, dtype=np.float32),
        "iotaf": np.ascontiguousarray(
            np.tile(np.arange(128, dtype=np.float32),
                    (128, 1)).astype(ml_dtypes.bfloat16)),
    }
    for k in ["sage1_wl", "sage1_wr", "sage2_wl", "sage2_wr", "gat_w",
              "cls_w1", "cls_w2"]:
        base[k] = np.ascontiguousarray(np.asarray(inputs[k],
                                                  dtype=np.float32))
    for k in ["sage1_bl", "sage2_bl", "bn1_gamma", "bn1_beta", "bn1_mean",
              "bn1_var", "bn2_gamma", "bn2_beta", "bn2_mean", "bn2_var",
              "gat_bias", "cls_b1", "cls_b2"]:
        base[k] = np.ascontiguousarray(
            np.asarray(inputs[k], dtype=np.float32).reshape(1, -1))
    base["att_src_row"] = np.ascontiguousarray(
        np.asarray(inputs["gat_att_src"], dtype=np.float32).reshape(1, -1))
    base["att_dst_row"] = np.ascontiguousarray(
        np.asarray(inputs["gat_att_dst"], dtype=np.float32).reshape(1, -1))

    in_maps = []
    x = xf
    for c in range(P):
        m = dict(base)
        m["xloc"] = np.ascontiguousarray(x[c * NLOC : (c + 1) * NLOC])
        m.update(per_core[c])
        in_maps.append(m)
    return in_maps


def run(inputs, trace=False, **kw):
    layout, per_core = preprocess(np.asarray(inputs["edge_index"]))
    nc = build(layout)
    in_maps = make_in_maps(inputs, per_core)
    res = run_bass_kernel_spmd(nc, in_maps, core_ids=list(range(P)),
                               trace=trace, **kw)
    emb = np.concatenate([r["emb"] for r in res.results], axis=0)
    logits = np.concatenate([r["logits"] for r in res.results], axis=0)
    return (emb, logits), res


def kernel(**inputs):
    (emb, logits), _ = run(inputs, trace=False)
    return emb, logits
